# revision 1
# baseline (speedup 1.0000x reference)
"""Trainium2 Bass kernel for nn_BlockDiagonalLRU.

Reference computation (B=4, T=1024, D=1024, H=64, M=16):
    h  = rmsnorm(x) * norm_w
    v  = (h @ W_v.T)                      [B,T,H,M]
    g  = softmax((h @ W_a.T).reshape(B,T,H,M,M+1), -1)
    a0 = g[...,0]; A = g[...,1:]
    s_t = A_t s_{t-1} + a0_t * v_t        (scan over T, per (b,h))
    out = x + ys @ W_out.T

Sharding: 8 cores, core c owns h in [8c, 8c+8).  Each core computes the
gates/v matmuls for its h-block over all (B,T), runs its 32 (b,h) scans,
and produces a partial output  ys_blk @ W_out[:, blk].T  which the host
sums across cores and adds to the residual x.

Device pipeline per core:
  fp32r matmuls (full PE rate at N=512) -> PSUM -> ACT exp evacuation
  with per-token rmsnorm scale r -> DVE grouped softmax denominator +
  reciprocal -> GPSIMD normalize -> DMA re-layout through a DRAM bounce
  into scan layout [(b,h) partitions, (s,i,j') free] -> sequential DVE scan
  (2 ops/step: broadcast-AP multiply + grouped reduce) -> ys re-layout
  to [(h,i), t] -> W_out matmul -> partial out.
"""

import contextlib
import os

import numpy as np

import concourse.bass as bass
import concourse.tile as tile
from concourse import bacc
from concourse import mybir
from concourse.bass_utils import run_bass_kernel_spmd

B, T, D = 4, 1024, 1024
M, MP1 = 16, 17
H = 64
EPS = 1e-5
NCORES = 8
HPC = H // NCORES          # 8 h per core
GW = M * MP1               # 272 gate cols per h
NG = HPC * GW              # 2176 gate cols per core
NV = HPC * M               # 128 v cols per core
NCOLS = NG + NV            # 2304 matmul cols per core
NK = D // 128              # 8 k tiles
NTT = T // 128             # 8 token tiles per b
F32 = mybir.dt.float32
F32R = mybir.dt.float32r
MULT = mybir.AluOpType.mult
ADD = mybir.AluOpType.add

# PSUM n-chunks over the 2304 matmul output cols
CHUNKS = [(0, 512), (512, 512), (1024, 512), (1536, 512), (2048, 256)]


def _emit(tc, nc, xT, xn, wcat, woutT, pout, gbounce, repeat=1):
    ctx = contextlib.ExitStack()
    with ctx:
        singles = ctx.enter_context(tc.tile_pool(name="singles", bufs=1))
        xtp = ctx.enter_context(tc.tile_pool(name="xtp", bufs=2))
        xnp = ctx.enter_context(tc.tile_pool(name="xnp", bufs=2))
        gpool = ctx.enter_context(tc.tile_pool(name="gpool", bufs=2))
        zpool = ctx.enter_context(tc.tile_pool(name="zpool", bufs=2))
        rpool = ctx.enter_context(tc.tile_pool(name="rpool", bufs=4))
        Rpool = ctx.enter_context(tc.tile_pool(name="Rpool", bufs=3))
        ypool = ctx.enter_context(tc.tile_pool(name="ypool", bufs=2))
        ptpool = ctx.enter_context(tc.tile_pool(name="ptpool", bufs=2))
        ytp = ctx.enter_context(tc.tile_pool(name="ytp", bufs=2))
        obuf = ctx.enter_context(tc.tile_pool(name="obuf", bufs=1))
        gpsum = ctx.enter_context(tc.tile_pool(name="gpsum", bufs=6, space="PSUM"))
        opsum = ctx.enter_context(tc.tile_pool(name="opsum", bufs=2, space="PSUM"))

        # ---- resident constants ----
        wcat_sb = []
        for k in range(NK):
            wk = singles.tile([128, NCOLS], F32R, tag=f"wcat{k}", name=f"wcat{k}")
            nc.sync.dma_start(out=wk, in_=wcat[k * 128 : (k + 1) * 128, :])
            wcat_sb.append(wk)
        woutT_sb = singles.tile([128, D], F32, tag="woutT", name="woutT_sb")
        nc.sync.dma_start(out=woutT_sb, in_=woutT[:, :])

        eps_t = singles.tile([128, 1], F32, tag="eps", name="eps_t")
        nc.vector.memset(eps_t, EPS)

        # scan init state column: [1, 0, ..., 0] per (b,h) row
        init_t = singles.tile([32, MP1], F32, tag="init", name="init_t")
        nc.vector.memset(init_t, 0.0)
        nc.vector.memset(init_t[:, 0:1], 1.0)

        # ---- phase 0: rmsnorm scales r for all (b, tt) (keeps ACT on Sqrt,
        # then the whole main loop stays on the Exp table set) ----
        r_all = singles.tile([128, B * NTT], F32, tag="rall", name="r_all")
        for b in range(B):
            for tt in range(NTT):
                idx = b * NTT + tt
                xt_ = xnp.tile([128, D], F32, tag="xn", name="xt_")
                nc.scalar.dma_start(
                    out=xt_, in_=xn[b, tt * 128 : (tt + 1) * 128, :]
                )
                st_ = rpool.tile([128, 2, 6], F32, tag="bnst", name="st_")
                for sg in range(2):
                    nc.vector.bn_stats(
                        out=st_[:, sg, :], in_=xt_[:, sg * 512 : (sg + 1) * 512]
                    )
                mv = rpool.tile([128, 2], F32, tag="bnmv", name="mv")
                nc.vector.bn_aggr(out=mv, in_=st_)
                rc = r_all[:, idx : idx + 1]
                # mean(x^2) = mean^2 + var
                nc.vector.scalar_tensor_tensor(
                    out=rc, in0=mv[:, 0:1], scalar=mv[:, 0:1], in1=mv[:, 1:2],
                    op0=MULT, op1=ADD,
                )
                nc.scalar.activation(
                    out=rc, in_=rc, func=mybir.ActivationFunctionType.Sqrt,
                    bias=eps_t, scale=1.0,
                )
                nc.vector.reciprocal(out=rc, in_=rc)

        # ys ring: two persistent tiles; row 0 is a constant 1.0 column
        # (the scan state vector is read as [1, s_1..s_16])
        ys_ring = []
        for ri in range(2):
            yt = ypool.tile([32, MP1, 128], F32, tag=f"ysr{ri}", name=f"ysr{ri}")
            nc.vector.memset(yt[:, 0:1, :], 1.0)
            ys_ring.append(yt)

        # ---- main pipeline (repeat>1 re-runs it for timing; identical output) ----
        pools = (xtp, gpool, zpool, Rpool, ypool, ptpool, ytp, obuf, gpsum, opsum)
        pools = pools + (ys_ring,)
        for _rep in range(repeat):
            _emit_main(tc, nc, pools, xT, wcat_sb, woutT_sb, pout, gbounce,
                       r_all, init_t)


def _emit_main(tc, nc, pools, xT, wcat_sb, woutT_sb, pout, gbounce, r_all, init_t):
    (xtp, gpool, zpool, Rpool, ypool, ptpool, ytp, obuf, gpsum, opsum,
     ys_ring) = pools
    if True:
        prev_ys = None          # previous token-tile's ys tile (scan carry)
        pending = None          # deferred W_out work: (ys tile, tt)

        for tt in range(NTT):
            for b in range(B):
                rc = r_all[:, b * NTT + tt : b * NTT + tt + 1]

                # one DMA for all 8 k-tiles: xk[p, k, t] = xT[b, k*128+p, tt*128+t]
                xk = xtp.tile([128, NK, 128], F32R, tag="xt", name="xk")
                src = bass.AP(
                    tensor=xT,
                    offset=b * D * T + tt * 128,
                    ap=[[T, 128], [128 * T, NK], [1, 128]],
                )
                nc.sync.dma_start(out=xk, in_=src)
                xts = [xk[:, k, :] for k in range(NK)]

                gates_t = gpool.tile([128, NCOLS], F32, tag="gates", name="gates_t")

                for c0, csz in CHUNKS:
                    ps = gpsum.tile([128, 512], F32, tag="gps", name="ps")
                    for k in range(NK):
                        nc.tensor.matmul(
                            ps[:, 0:csz],
                            lhsT=xts[k],
                            rhs=wcat_sb[k][:, c0 : c0 + csz],
                            start=(k == 0),
                            stop=(k == NK - 1),
                        )
                    if c0 + csz <= NG:
                        nc.scalar.activation(
                            out=gates_t[:, c0 : c0 + csz], in_=ps[:, 0:csz],
                            func=mybir.ActivationFunctionType.Exp,
                            bias=0.0, scale=rc,
                        )
                    else:
                        gtail = NG - c0
                        nc.scalar.activation(
                            out=gates_t[:, c0:NG], in_=ps[:, 0:gtail],
                            func=mybir.ActivationFunctionType.Exp,
                            bias=0.0, scale=rc,
                        )
                        nc.scalar.activation(
                            out=gates_t[:, NG:NCOLS], in_=ps[:, gtail:csz],
                            func=mybir.ActivationFunctionType.Identity,
                            bias=0.0, scale=rc,
                        )

                # softmax denominator per 17-group and normalize
                gview = gates_t[:, 0:NG].rearrange("p (h i j) -> p h i j", i=M, j=MP1)
                z_t = zpool.tile([128, NV], F32, tag="z", name="z_t")
                nc.vector.tensor_reduce(
                    out=z_t, in_=gview, axis=mybir.AxisListType.X, op=ADD
                )
                rz_t = zpool.tile([128, NV], F32, tag="rz", name="rz_t")
                nc.vector.reciprocal(out=rz_t, in_=z_t)
                rz_b = (
                    rz_t.rearrange("p (h i) -> p h i", i=M)
                    .unsqueeze(3)
                    .broadcast_to([128, HPC, M, MP1])
                )
                nc.gpsimd.tensor_tensor(out=gview, in0=gview, in1=rz_b, op=MULT)

                # u = a0 * v written into the j'=0 slots
                j0 = gview[:, :, :, 0]
                vv = gates_t[:, NG:NCOLS].rearrange("p (h i) -> p h i", i=M)
                nc.vector.tensor_mul(j0, j0, vv)

                # bounce the gate region to DRAM, stored as [tt][b][h][t][col]
                # so the scan-layout load below merges (b, h) into one dim
                gb_off = (tt * B + b) * 128 * NG
                gb_dst = bass.AP(
                    tensor=gbounce,
                    offset=gb_off,
                    ap=[[GW, 128], [128 * GW, HPC], [1, GW]],
                )
                nc.scalar.dma_start(out=gb_dst, in_=gates_t[:, 0:NG])

            # load scan-layout pieces from the DRAM bounce buffer:
            # piece[p][b*8+h, s, :] = gbounce[tt, b, h, 16*p + s, :]
            pieces = []
            for p in range(8):
                Rp = Rpool.tile([32, 16, GW], F32, tag="R", name="Rp")
                src = bass.AP(
                    tensor=gbounce,
                    offset=tt * B * 128 * NG + p * 16 * GW,
                    ap=[[128 * GW, 32], [GW, 16], [1, GW]],
                )
                nc.sync.dma_start(out=Rp, in_=src)
                pieces.append(Rp)

            # deferred W_out matmuls for the previous token tile
            if pending is not None:
                _emit_wout(nc, ytp, obuf, opsum, woutT_sb, pout, *pending)

            # ---- scan this token tile (all 4 b in parallel on partitions) ----
            ys_t = ys_ring[tt % 2]
            for s in range(128):
                in0 = pieces[s // 16][:, s % 16, :].rearrange(
                    "p (i j) -> p i j", j=MP1
                )
                if s == 0:
                    src = init_t if prev_ys is None else prev_ys[:, :, 127]
                else:
                    src = ys_t[:, :, s - 1]
                in1 = src.unsqueeze(1).broadcast_to([32, M, MP1])
                pt = ptpool.tile([32, M, MP1], F32, tag="pt", name="pt")
                nc.vector.tensor_tensor(out=pt, in0=in0, in1=in1, op=MULT)
                nc.vector.tensor_reduce(
                    out=ys_t[:, 1:MP1, s], in_=pt,
                    axis=mybir.AxisListType.X, op=ADD,
                )
            prev_ys = ys_t
            pending = (ys_t, tt)

        _emit_wout(nc, ytp, obuf, opsum, woutT_sb, pout, *pending)


def _emit_wout(nc, ytp, obuf, opsum, woutT_sb, pout, ys_t, tt):
    for b in range(B):
        ysT = ytp.tile([128, 128], F32, tag="ysT", name="ysT")
        nc.sync.dma_start(out=ysT, in_=ys_t[b * HPC : (b + 1) * HPC, 1:MP1, :])
        o_sb = obuf.tile([128, D], F32, tag="osb", name="o_sb")
        for n in range(2):
            ps = opsum.tile([128, 512], F32, tag="ops", name="ps2")
            nc.tensor.matmul(
                ps,
                lhsT=ysT,
                rhs=woutT_sb[:, n * 512 : (n + 1) * 512],
                start=True,
                stop=True,
            )
            nc.scalar.copy(out=o_sb[:, n * 512 : (n + 1) * 512], in_=ps)
        nc.sync.dma_start(out=pout[b, tt * 128 : (tt + 1) * 128, :], in_=o_sb)


def _build_program(repeat=1):
    nc = bacc.Bacc()
    xT = nc.dram_tensor("xT", [B, D, T], F32R, kind="ExternalInput")
    xn = nc.dram_tensor("xn", [B, T, D], F32, kind="ExternalInput")
    wcat = nc.dram_tensor("wcat", [D, NCOLS], F32R, kind="ExternalInput")
    woutT = nc.dram_tensor("woutT", [HPC * M, D], F32, kind="ExternalInput")
    pout = nc.dram_tensor("pout", [B, T, D], F32, kind="ExternalOutput")
    gbounce = nc.dram_tensor("gbounce", [B * NTT * 128 * NG], F32)
    with tile.TileContext(nc) as tc:
        _emit(tc, nc, xT, xn, wcat, woutT, pout, gbounce, repeat=repeat)
    nc.finalize()
    return nc


_NC_CACHE = None


def _get_program():
    global _NC_CACHE
    rep = int(os.environ.get("KERNEL_REPEAT", "1"))
    if _NC_CACHE is None or _NC_CACHE[1] != rep:
        _NC_CACHE = (_build_program(repeat=rep), rep)
    return _NC_CACHE[0]


def make_in_maps(x, norm_w, W_v, W_a, W_out):
    """Host-side prep: fold norm_w into weights, shard per core."""
    x = np.asarray(x, dtype=np.float32)
    Wv_s = (np.asarray(W_v, np.float32) * norm_w[None, :]).reshape(H, M, D)
    Wa_s = (np.asarray(W_a, np.float32) * norm_w[None, :]).reshape(H, M, MP1, D)
    W_out = np.asarray(W_out, np.float32)
    xT = np.ascontiguousarray(np.swapaxes(x, 1, 2))   # [B, D, T]

    in_maps = []
    for c in range(NCORES):
        h0 = c * HPC
        ga = Wa_s[h0 : h0 + HPC].reshape(HPC * M * MP1, D)
        vv = Wv_s[h0 : h0 + HPC].reshape(HPC * M, D)
        wcat = np.ascontiguousarray(np.concatenate([ga, vv], axis=0).T)
        woutT = np.ascontiguousarray(W_out[:, h0 * M : (h0 + HPC) * M].T)
        in_maps.append({"xT": xT, "xn": x, "wcat": wcat, "woutT": woutT})
    return in_maps


def kernel(x, norm_w, W_v, W_a, W_out):
    x = np.asarray(x, dtype=np.float32)
    in_maps = make_in_maps(x, np.asarray(norm_w, np.float32), W_v, W_a, W_out)
    nc = _get_program()
    res = run_bass_kernel_spmd(
        nc,
        in_maps,
        list(range(NCORES)),
        trace=bool(int(os.environ.get("KERNEL_TRACE", "0"))),
    )
    if res.exec_time_ns is not None:
        print(f"HW exec time: {res.exec_time_ns} ns")

    out = x.copy()
    for c in range(NCORES):
        out += res.results[c]["pout"]
    return out

